# revision 45
# baseline (speedup 1.0000x reference)
"""Bass/Trainium2 kernel for nn_DotProductAttention_47528108097846.

reference:
    scores = einsum('bhqd,bhkd->bhqk', Q, K) / 16
    attn = softmax(scores, axis=-1)
    h = einsum('bhqk,bhkd->bhqd', attn, V)
    return reshape(h, (S, B, H, D))

B=2, H=8, S=4096, D=64. 16 (b,h) pairs sharded as 2 per NeuronCore across 8
cores (batch+head parallel, no cross-core comms).

Per-core algorithm (2 heads), all matmuls bf16 (weights zero-padded to 128
rows/cols so FastWeightLoad hides the per-matmul weight reload; accumulation
is always fp32 in PSUM):
  - PE-transpose Q,K into QT,KT [64, 4096] bf16, build V' = [V | 1 | 0pad]
    [128, 128] per 128-wide k-block.
  - For each 1024-wide q-group, for each k-block kb:
      scoresT[kb] [128,1024] = (lhsT=KT_kb).T @ (rhs=QT slice)   (PSUM fp32)
      expT = exp(scoresT / 16)            (ScalarE, scale fused, no max-sub:
                                           |scores| <= ~4 for randn inputs)
      outT [128,1024] += (lhsT=V'_kb).T @ expT   (accumulating matmul; row 64
                                           = sum of exp = softmax denominator)
  - Transpose outT in [65,128] strips to [128,65], multiply cols 0:64 by
    reciprocal of col 64 (DVE), DMA to DRAM.

Measured on trn2: ~334 us/core device time, l2 rel err 2.7e-3.
The kernel is ScalarE-bound: 33.6M exps/core at 1 el/lane/cycle @1.2GHz
(284 us) with PE at ~279 us hidden underneath.
"""
import numpy as np

import concourse.bass as bass
import concourse.bacc as bacc
import concourse.tile as tile
from concourse import mybir
from concourse.masks import make_identity
from concourse.bass_utils import run_bass_kernel_spmd

B, H, S, D = 2, 8, 4096, 64
N_CORES = 8
PAIRS_PER_CORE = (B * H) // N_CORES  # 2 heads per core

f32 = mybir.dt.float32
f32r = mybir.dt.float32r
bf16 = mybir.dt.bfloat16

QG = 1024            # q-group width (psum scores buffer = QG*4B = 2 banks)
NQG = S // QG        # 4 q-groups per head
NKB = S // 128       # 32 k-blocks per head


def build_attention(nc, tc, q, k, v, o, qk_dtype=bf16, av_dtype=bf16,
                    repeat_loop=None, mode="full"):
    """Emit attention for PAIRS_PER_CORE heads.

    q/k/v/o: DRAM APs of shape [PAIRS_PER_CORE, S, D] (fp32).
    repeat_loop: if not None, wrap the main compute in For_i(0, repeat_loop)
    for benchmarking.
    """
    import contextlib
    ctx = contextlib.ExitStack()
    consts = ctx.enter_context(tc.tile_pool(name="consts", bufs=1))
    nat = ctx.enter_context(tc.tile_pool(name="nat", bufs=2))
    persist = ctx.enter_context(tc.tile_pool(name="persist", bufs=1))
    sb = ctx.enter_context(tc.tile_pool(name="sb", bufs=3))
    pool_e = ctx.enter_context(tc.tile_pool(name="sb_e", bufs=6))
    pool_s = ctx.enter_context(tc.tile_pool(name="ps_s", bufs=2, space="PSUM"))
    pool_o = ctx.enter_context(tc.tile_pool(name="ps_o", bufs=1, space="PSUM"))
    pool_t = ctx.enter_context(tc.tile_pool(name="ps_t", bufs=2, space="PSUM"))

    if mode == "copyonly":
        for h in range(PAIRS_PER_CORE):
            t = None
            for src in (q, k, v):
                t = nat.tile([128, NKB, 64], f32, tag="nat")
                nc.sync.dma_start(
                    out=t, in_=src[h].rearrange("(n p) d -> p n d", p=128))
            nc.sync.dma_start(
                out=o[h].rearrange("(n p) d -> p n d", p=128), in_=t)
        ctx.close()
        return

    ident = consts.tile([128, 128], f32)
    make_identity(nc, ident)
    identb = consts.tile([128, 128], qk_dtype)
    nc.vector.tensor_copy(out=identb, in_=ident)
    ones128 = consts.tile([128, 1], f32)
    nc.vector.memset(ones128, 1.0)
    zero128 = consts.tile([128, 1], f32)
    nc.vector.memset(zero128, 0.0)

    # ---------------- prologue: load + transpose Q,K; build V' ----------
    # QT/KT padded to 128 contraction rows (rows 64.. are zero) and V' padded
    # to 128 columns (cols 65.. zero) so every matmul carries full 128-wide
    # bf16 weights -> FastWeightLoad can hide the per-matmul weight reload.
    qts, kts, v1s = [], [], []

    def emit_prologue(h):
        qt = persist.tile([128, NKB, 128], qk_dtype, tag=f"qt{h}")
        kt = persist.tile([128, NKB, 128], qk_dtype, tag=f"kt{h}")
        v1 = persist.tile([128, NKB, 128], av_dtype, tag=f"v1{h}")
        qts.append(qt)
        kts.append(kt)
        v1s.append(v1)
        nc.gpsimd.memset(qt[64:128], 0.0)
        nc.gpsimd.memset(kt[64:128], 0.0)

        # interleave K/Q chunk loads+transposes (K first) so the first QK
        # matmul and first exp can start as early as possible
        CH = 8
        for g in range(NKB // CH):
            for (src, dst) in ((k, kt), (q, qt)):
                natc = nat.tile([128, CH, 64], f32, tag="nat")
                nc.sync.dma_start(
                    out=natc,
                    in_=src[h].rearrange("(n p) d -> p n d", p=128)[
                        :, g * CH:(g + 1) * CH, :])
                natbc = nat.tile([128, CH, 64], qk_dtype, tag="natb")
                nc.vector.tensor_copy(out=natbc, in_=natc)
                ps_tr = pool_t.tile([64, CH, 128], qk_dtype, tag="t")
                for j in range(CH):
                    nc.tensor.transpose(ps_tr[:, j, :], natbc[:, j, :], identb)
                nc.vector.tensor_copy(
                    out=dst[0:64, g * CH:(g + 1) * CH, :], in_=ps_tr)
            if g == 2:
                # V' build deferred past the first K/Q chunks so its DVE
                # copies don't delay the casts feeding the first QK matmuls;
                # ones column + zero pad go on idle GpSimd (bf16 memset)
                nc.gpsimd.memset(v1[:, :, 64:65], 1.0)
                nc.gpsimd.memset(v1[:, :, 65:128], 0.0)
                vnat = nat.tile([128, NKB, 64], f32, tag="vnat")
                nc.sync.dma_start(
                    out=vnat, in_=v[h].rearrange("(n p) d -> p n d", p=128))
                nc.vector.tensor_copy(out=v1[:, :, 0:64], in_=vnat)

    # head 0 upfront; later heads' prologues are emitted inside head 0's
    # main loop (after its first q-group) so their PE-transpose bursts
    # spread out instead of starving ScalarE early on
    emit_prologue(0)
    defer_prologues = (repeat_loop is None and mode == "full")
    if not defer_prologues:
        for h in range(1, PAIRS_PER_CORE):
            emit_prologue(h)

    # ---------------- main loops --------------------------------------
    def main_compute():
        for h in range(PAIRS_PER_CORE):
            qt, kt, v1 = qts[h], kts[h], v1s[h]
            out_r = o[h].rearrange("(n p) d -> p n d", p=128)
            for qg in range(NQG):
                ps_o = pool_o.tile([128, QG], f32, tag="o")

                def av(prev_eT, prev_kb, j):
                    nc.tensor.matmul(
                        out=ps_o[:, j * 512:(j + 1) * 512],
                        lhsT=v1[:, prev_kb, :],
                        rhs=prev_eT[:, j * 512:(j + 1) * 512],
                        start=(prev_kb == 0), stop=(prev_kb == NKB - 1))

                # software-pipelined: QK(kb) matmuls interleaved with the
                # accumulating AV matmuls of kb-1, so every start/stop QK
                # matmul's pipe drain hides under an adjacent AV fill
                prev = None
                for kb in range(NKB):
                    ps_s = pool_s.tile([128, QG], f32, tag="s")
                    for j in range(QG // 512):
                        nc.tensor.matmul(
                            out=ps_s[:, j * 512:(j + 1) * 512],
                            lhsT=kt[:, kb, :],
                            rhs=qt.rearrange("p n d -> p (n d)")[
                                :, qg * QG + j * 512: qg * QG + (j + 1) * 512],
                            start=True, stop=True)
                        if prev is not None:
                            av(prev[0], prev[1], j)
                    eT = pool_e.tile([128, QG], av_dtype, tag="exp")
                    nc.scalar.activation(
                        out=eT, in_=ps_s,
                        func=mybir.ActivationFunctionType.Exp,
                        scale=1.0 / 16.0)
                    prev = (eT, kb)
                for j in range(QG // 512):
                    av(prev[0], prev[1], j)
                # epilogue for this q-group
                oT = sb.tile([65, QG], f32, tag="oT")
                nc.vector.tensor_copy(out=oT, in_=ps_o[0:65, :])
                out_sb = sb.tile([128, QG // 128, 64], f32, tag="out")
                for i in range(QG // 128):
                    ps_t = pool_t.tile([128, 65], f32, tag="t")
                    nc.tensor.transpose(
                        ps_t, oT[:, i * 128:(i + 1) * 128],
                        ident[0:65, 0:65])
                    rcp = sb.tile([128, 1], f32, tag="rcp")
                    nc.vector.reciprocal(out=rcp, in_=ps_t[:, 64:65])
                    nc.vector.tensor_scalar_mul(
                        out=out_sb[:, i, :], in0=ps_t[:, 0:64], scalar1=rcp)
                nc.sync.dma_start(
                    out=out_r[:, qg * (QG // 128):(qg + 1) * (QG // 128), :],
                    in_=out_sb)
                if defer_prologues and h == 0 and qg == 0:
                    for h2 in range(1, PAIRS_PER_CORE):
                        emit_prologue(h2)

    if mode == "prologue":
        pass
    elif repeat_loop is None:
        main_compute()
    else:
        with tc.For_i(0, repeat_loop, 1) as _:
            main_compute()
    ctx.close()


_CACHED = {}


def build_program(qk_dtype=bf16, av_dtype=bf16, repeat_loop=None, mode="full"):
    key = (str(qk_dtype), str(av_dtype), repeat_loop, mode)
    if key in _CACHED:
        return _CACHED[key]
    nc = bacc.Bacc("TRN2", target_bir_lowering=False, debug=False,
                   num_devices=N_CORES)
    q = nc.dram_tensor("q", [PAIRS_PER_CORE, S, D], f32,
                       kind="ExternalInput").ap()
    k = nc.dram_tensor("k", [PAIRS_PER_CORE, S, D], f32,
                       kind="ExternalInput").ap()
    v = nc.dram_tensor("v", [PAIRS_PER_CORE, S, D], f32,
                       kind="ExternalInput").ap()
    o = nc.dram_tensor("o", [PAIRS_PER_CORE, S, D], f32,
                       kind="ExternalOutput").ap()
    with tile.TileContext(nc) as tc:
        build_attention(nc, tc, q, k, v, o, qk_dtype=qk_dtype,
                        av_dtype=av_dtype, repeat_loop=repeat_loop, mode=mode)
    nc.compile()
    _CACHED[key] = nc
    return nc


def kernel(queries, keys, values, adj=None, **_unused):
    """Full-input attention on 8 NeuronCores. Returns [S, B, H, D] fp32."""
    queries = np.ascontiguousarray(queries, dtype=np.float32)
    keys = np.ascontiguousarray(keys, dtype=np.float32)
    values = np.ascontiguousarray(values, dtype=np.float32)

    nc = build_program()
    qf = queries.reshape(B * H, S, D)
    kf = keys.reshape(B * H, S, D)
    vf = values.reshape(B * H, S, D)
    in_maps = []
    for c in range(N_CORES):
        sl = slice(c * PAIRS_PER_CORE, (c + 1) * PAIRS_PER_CORE)
        in_maps.append({"q": qf[sl], "k": kf[sl], "v": vf[sl]})
    res = run_bass_kernel_spmd(nc, in_maps, list(range(N_CORES)))
    hout = np.empty((B * H, S, D), dtype=np.float32)
    for c in range(N_CORES):
        hout[c * PAIRS_PER_CORE:(c + 1) * PAIRS_PER_CORE] = res.results[c]["o"]
    return hout.reshape(B, H, S, D).reshape(S, B, H, D)
